# revision 82
# baseline (speedup 1.0000x reference)
"""GNN mean-aggregation message passing on 8 Trainium2 NeuronCores.

out[t] = mean_{e: tgt[e]==t} x[src[e]]   (0 if no incoming edges)

Strategy (target-sharded, uniform SPMD schedule):
  - Targets are dealt to cores serpentine-by-degree so per-(core,group) edge
    counts are balanced; each core owns 12544 output slots (98 groups of 128).
  - Host packs x as bf16 hi|lo pairs -> [N_pad, 128] bf16 (256B rows). The
    hi+lo split recovers ~fp32 precision after the f32 PSUM accumulation.
  - Edges are routed to the owning core, ordered by (supergroup, src-chunk,
    target) and packed into 128-edge slots. Every slot is bound (at compile
    time, uniformly across cores) to a target group g; its edges may only
    reference groups {g, g+1}. One matmul per slot (plus one per spill slot)
    accumulates into per-group PSUM.
  - The one-hot selection matrices are PRECOMPUTED ON HOST (pure index
    metadata) as fp8 and streamed from HBM on the scalar-engine HWDGE ring:
    no on-device is_equal compares at all (frees the DVE).
  - Sources are gathered straight from HBM with dma_gather (int16 indices,
    4 chunks of 25088 rows to fit the int16 range). SWDGE descriptor
    emission (~8ns/idx per Q7 core pair) is the wall, so gathers round-robin
    all 4 SWDGE queues (queue q runs on Q7 pair {2q,2q+1}) for ~4x parallel
    emission; bins are split into equal <=GCALL-slot calls so the ~4-deep
    Pool dispatch window stays in lockstep.
  - Supergroups are 13 groups (4 PSUM banks): consecutive supergroups
    double-buffer in the 8 banks, so finalize never barriers the pipeline.
  - idx is loaded per supergroup (separate tiles: no false WAR dep, few DMA
    sem lanes); out is [partition, group*F] so the store is 16 big
    descriptors per supergroup instead of 128*gs small ones.
  - Finalize per group: (psum_hi + psum_lo) * inv_count -> DMA out.

Measured on the 8-core axon pod: 1642390 ns (baseline) -> ~480000 ns.
"""
import sys

sys.path.insert(0, "/opt/trn_rl_repo")

import numpy as np
import ml_dtypes

bf16 = ml_dtypes.bfloat16

# ---- problem constants (hardcoded per harness contract) ----
N, F, E = 100000, 64, 1600000
P = 128
NCORES = 8
TPC = 12544                  # target slots per core (= 98 * 128)
GPC = TPC // P               # 98 groups per core
NCHUNKS = 4
CHUNK = 25088                # source rows per chunk (< 32768 for int16 idx)
NPAD = NCHUNKS * CHUNK       # 100352 padded source rows
SG_SIZES = [13, 13, 13, 13, 13, 13, 13, 6, 1]   # 4 PSUM banks per sg ->
# consecutive supergroups double-buffer in the 8 banks (no finalize barrier);
# tiny last one keeps the matmul tail short
ELEM = 2 * F                 # 128 bf16 per packed row = 256B
SENT = 384.0                 # sentinel row id for pad edges (no iota match)
SEL_PIECE = 16               # spill-sel window (columns per spill DMA load)
GCALL = 28                   # slots per dma_gather call: bins split across two
                             # queues so per-bin emission latency halves
NQUEUES = 4                  # SWDGE queues; queue q emits on Q7 core pair {2q,2q+1}
SINGLE_PACKET = False        # True (>64-desc packets) faults the DMA engines
DMA_SCRATCH = 16384          # SWDGE ring carveout bytes/partition


def _chunk_order(sched, s):
    return sorted(range(NCHUNKS), key=lambda c: (-len(sched[s][c]), c))


def _call_sizes(w):
    """Split a bin of w slots into equal-ish gather calls of <=GCALL slots."""
    ncalls = -(-w // GCALL)
    base, rem = divmod(w, ncalls)
    return [base + 1] * rem + [base] * (ncalls - rem)


def _call_plan(sched):
    """Gather-call list in emission order: (slot0, nslots, queue, sg).

    Single source of truth for host idx packing and the device program.
    Queue-packed idx layout: queue q only reads partitions [32q, 32q+32),
    so each call's idx lives only in its own queue's partition group.
    Column offsets: per queue, sg0-1 calls pack from col 0; later calls
    pack from col CA (the max sg0-1 extent) so the two idx tiles split
    cleanly at CA with no call straddling.
    """
    plan = []
    so = 0
    q = 0
    for s in range(len(SG_SIZES)):
        for c in _chunk_order(sched, s):
            for gw in _call_sizes(len(sched[s][c])):
                plan.append([so, gw, q % NQUEUES, s])
                q += 1
                so += gw
    offs = [0] * NQUEUES
    colsA = [0] * NQUEUES
    for p in plan:
        if p[3] <= 1:
            p.append(offs[p[2]])
            offs[p[2]] += p[1] * 8
            colsA[p[2]] = offs[p[2]]
    ca = max(colsA)
    offs = [ca] * NQUEUES
    for p in plan:
        if p[3] > 1:
            p.append(offs[p[2]])
            offs[p[2]] += p[1] * 8
    return plan, ca, max(offs)


def _balance_targets(tgt):
    """Serpentine-by-degree target -> (core, rank) assignment.

    Returns perm_o2n[old_target] = new_target_id (core*TPC + rank)."""
    deg = np.bincount(tgt, minlength=N)
    order = np.argsort(-deg, kind="stable")
    perm = np.empty(N, np.int64)
    pos = np.arange(N)
    rows = pos // NCORES
    cols = pos % NCORES
    cores = np.where(rows % 2 == 0, cols, NCORES - 1 - cols)
    for k in range(NCORES):
        ids = order[cores == k]
        perm[ids] = k * TPC + np.arange(ids.size)
    return perm


def _host_prep(x, edge_idx):
    """Build per-core device arrays and the shared slot schedule."""
    x = np.asarray(x, np.float32)
    src = np.asarray(edge_idx[0], np.int64)
    tgt_orig = np.asarray(edge_idx[1], np.int64)

    perm = _balance_targets(tgt_orig)
    tgt = perm[tgt_orig]

    # packed hi|lo bf16 table
    hi = x.astype(bf16)
    lo = (x - hi.astype(np.float32)).astype(bf16)
    xp = np.zeros((NPAD, ELEM), bf16)
    xp[:N, :F] = hi
    xp[:N, F:] = lo

    inv_cnt = np.bincount(tgt, minlength=NCORES * TPC).astype(np.float32)
    inv_cnt = 1.0 / np.maximum(inv_cnt, 1.0)

    core = tgt // TPC
    chunk = src // CHUNK
    gl = (tgt // P) - core * GPC          # local group 0..97
    sg_bounds = np.cumsum([0] + SG_SIZES)
    sg = np.searchsorted(sg_bounds, gl, side="right") - 1
    order = np.lexsort((tgt, chunk, sg, core))

    nsg = len(SG_SIZES)
    bin_id = (core * nsg + sg) * NCHUNKS + chunk
    bin_sizes = np.bincount(bin_id, minlength=NCORES * nsg * NCHUNKS)
    bin_starts = np.zeros(NCORES * nsg * NCHUNKS + 1, np.int64)
    np.cumsum(bin_sizes, out=bin_starts[1:])

    gl_sorted = gl[order]
    tgt_sorted = tgt[order]
    src_sorted = src[order]
    chunk_sorted = chunk[order]

    # ---- build shared schedule: per (sg, c) the block label list ----
    sched = []          # sched[sg][c] = np.array of block labels h (bin-local)
    for s in range(nsg):
        gs = SG_SIZES[s]
        row = []
        for c in range(NCHUNKS):
            e_kh = np.zeros((NCORES, gs), np.int64)
            for k in range(NCORES):
                b = (k * nsg + s) * NCHUNKS + c
                seg = gl_sorted[bin_starts[b]:bin_starts[b + 1]] - sg_bounds[s]
                if seg.size:
                    e_kh[k] = np.bincount(seg, minlength=gs)
            labels = []
            r = e_kh[:, 0].astype(np.int64)
            for h in range(gs):
                s_h = int(np.ceil(r / P).max())
                labels.extend([h] * s_h)
                cap = s_h * P - r
                if h + 1 < gs:
                    r = np.maximum(0, e_kh[:, h + 1] - cap)
                else:
                    assert (cap >= 0).all()
            row.append(np.asarray(labels, np.int64))
        sched.append(row)

    tot_slots = sum(len(row[c]) for row in sched for c in range(NCHUNKS))
    tot = tot_slots * P

    # ---- per-core edge placement into the uniform slot stream ----
    src_local = np.zeros((NCORES, tot), np.int16)
    trow = np.full((NCORES, tot), SENT, np.float32)
    spill = np.zeros(tot_slots, bool)   # slot has any lane in group h+1
    for k in range(NCORES):
        base = 0
        for s in range(nsg):
            for c in _chunk_order(sched, s):
                labels = sched[s][c]
                b = (k * nsg + s) * NCHUNKS + c
                lo_i, hi_i = bin_starts[b], bin_starts[b + 1]
                garr = gl_sorted[lo_i:hi_i] - sg_bounds[s]
                p = 0
                for bi, h in enumerate(labels):
                    upper = np.searchsorted(garr, h + 1, side="right")
                    take = min(P, upper - p)
                    if take > 0:
                        sl = slice(lo_i + p, lo_i + p + take)
                        pos = base + bi * P
                        src_local[k, pos:pos + take] = (
                            src_sorted[sl] - chunk_sorted[sl] * CHUNK
                        ).astype(np.int16)
                        trow[k, pos:pos + take] = (
                            tgt_sorted[sl] % P + P * (garr[p:p + take] - h)
                        ).astype(np.float32)
                        if (garr[p:p + take] > h).any():
                            spill[base // P + bi] = True
                        p += take
                assert p == hi_i - lo_i, (
                    f"core {k} sg {s} c {c}: placed {p} of {hi_i - lo_i}"
                )
                base += len(labels) * P
        assert base == tot

    # device layouts: idx queue-packed — call idx only in its queue's
    # partition group [32q, 32q+32) (16-row wrap, duplicated for the pair's
    # two cores); other groups hold other calls' data (ignored by the ucode)
    plan, ca, ctot = _call_plan(sched)
    idx_dev = []
    for k in range(NCORES):
        arr = np.zeros((P, ctot), np.int16)
        for slot0, ns, q, _sg, off in plan:
            blk = (src_local[k, slot0 * P:(slot0 + ns) * P]
                   .reshape(ns * 8, 16).T)
            arr[32 * q:32 * q + 16, off:off + ns * 8] = blk
            arr[32 * q + 16:32 * q + 32, off:off + ns * 8] = blk
        idx_dev.append(arr)
    # sel one-hot matrices precomputed on host (pure index metadata) and
    # streamed from HBM as fp8: kills the DVE is_equal compares entirely.
    # main: [lane, slot, col] 1.0 where col == tgt%128 within the slot's own
    # group; spill: compact columns, one per spill slot (shared slot set).
    fp8 = ml_dtypes.float8_e4m3
    spill_slots = np.flatnonzero(spill)               # shared across cores
    n_spill = len(spill_slots)
    nsp_pad = (n_spill + SEL_PIECE - 1) // SEL_PIECE * SEL_PIECE
    nslots = tot // P
    lanes = np.tile(np.arange(P), nslots)             # lane of each stream pos
    slots = np.repeat(np.arange(nslots), P)
    selm_dev = []
    selsp_dev = []
    for k in range(NCORES):
        v = trow[k].astype(np.int64)                  # [tot] 0..255 / SENT
        selm = np.zeros((P, nslots, P), fp8)
        m = v < P
        selm[lanes[m], slots[m], v[m]] = 1.0
        sp = np.zeros((P, nsp_pad, P), fp8)
        vs = v.reshape(nslots, P)[spill_slots]        # [n_spill, lane]
        li, si_ = np.nonzero((vs.T >= P) & (vs.T < 2 * P))
        sp[li, si_, vs.T[li, si_] - P] = 1.0
        selm_dev.append(selm.reshape(P, nslots * P))
        selsp_dev.append(sp.reshape(P, nsp_pad * P))
    invc_dev = [
        inv_cnt[k * TPC:(k + 1) * TPC].reshape(GPC, P).T.copy()
        for k in range(NCORES)
    ]
    return (xp, idx_dev, selm_dev, selsp_dev, invc_dev, sched, tot,
            spill, perm)


def _build_program(sched, tot, spill):
    from concourse import bacc, mybir, tile

    nsg = len(SG_SIZES)
    gsg_max = max(SG_SIZES)
    spill_cols = np.cumsum(spill) - spill      # slot -> its trowsp column
    n_spill = int(spill.sum())
    nsp_pad = (n_spill + SEL_PIECE - 1) // SEL_PIECE * SEL_PIECE

    nc = bacc.Bacc(None, target_bir_lowering=False, debug=False,
                   num_swdge_queues=NQUEUES,
                   dynamic_dma_scratch_size=DMA_SCRATCH)
    plan, ca, ctot = _call_plan(sched)
    t_x = nc.dram_tensor("xp", [NPAD, ELEM], mybir.dt.bfloat16, kind="ExternalInput")
    t_idx = nc.dram_tensor("idx", [P, ctot], mybir.dt.int16, kind="ExternalInput")
    t_selm = nc.dram_tensor("selm", [P, (tot // P) * P], mybir.dt.float8e4, kind="ExternalInput")
    t_selsp = nc.dram_tensor("selsp", [P, nsp_pad * P], mybir.dt.float8e4, kind="ExternalInput")
    t_invc = nc.dram_tensor("invc", [P, GPC], mybir.dt.float32, kind="ExternalInput")
    # out laid out [partition, group*F] so the finalize DMA is one contiguous
    # run per partition (16 big descriptors/call instead of 128*gs small ones)
    t_out = nc.dram_tensor("out", [P, GPC * F], mybir.dt.float32, kind="ExternalOutput")

    with tile.TileContext(nc) as tc:
        with (
            tc.tile_pool(name="const", bufs=1) as cpool,
            tc.tile_pool(name="msgs", bufs=9) as mpool,
            tc.tile_pool(name="sel", bufs=2) as spool,
            tc.tile_pool(name="spsel", bufs=2) as sppool,
            tc.tile_pool(name="stage", bufs=2) as stpool,
            tc.tile_pool(name="psum", bufs=8, space="PSUM") as ppool,
        ):
            # idx in two tiles split at column CA (sg0-1 calls | rest):
            # separate tiles avoid a false WAR dep, two loads keep the DMA
            # sem lanes quiet, and the queue-packed layout (each call's idx
            # only in its queue's 32-partition group) makes both tiles 4x
            # smaller than a fully replicated layout.
            idxa_t = cpool.tile([P, ca], mybir.dt.int16)
            idxb_t = cpool.tile([P, ctot - ca], mybir.dt.int16)
            nc.sync.dma_start(out=idxa_t[:], in_=t_idx[:, :ca])
            nc.sync.dma_start(out=idxb_t[:], in_=t_idx[:, ca:])

            def idx_ap(off, width):
                if off + width <= ca:
                    return idxa_t[:, off:off + width]
                assert off >= ca
                return idxb_t[:, off - ca:off - ca + width]

            invc_t = cpool.tile([P, GPC], mybir.dt.float32)
            nc.sync.dma_start(out=invc_t[:], in_=t_invc[:])
            call_iter = iter(plan)

            slot_off = 0     # global slot offset in the stream
            g_base = 0       # global group offset
            sp_state = [None, 0, 0]   # [tile, base_col, width] rolling spill sel
            for s in range(nsg):
                gs = SG_SIZES[s]
                nslots_psum = gs + 1
                nbanks = (nslots_psum + 3) // 4
                pts = [
                    ppool.tile([P, 4 * P], mybir.dt.float32, name=f"ps{s}_{b}", tag="ps")
                    for b in range(nbanks)
                ]
                for pt in pts:
                    nc.vector.memset(pt[:], 0.0)

                def pslot(j):
                    return pts[j // 4][:, (j % 4) * P:(j % 4 + 1) * P]

                for c in _chunk_order(sched, s):
                    labels = sched[s][c]
                    w = len(labels)
                    if w == 0:
                        continue
                    msgs_t = mpool.tile([P, w, ELEM], mybir.dt.bfloat16, name="msgs")
                    # split bins into EQUAL <=GCALL-slot gather calls (equal
                    # sizes keep the ~4-deep Pool dispatch window in lockstep;
                    # much above ~8K idx/call hits SWDGE ring-wrap stalls).
                    # Round-robin the 4 SWDGE queues so descriptor emission
                    # runs on all 4 Q7 core pairs concurrently.
                    g0 = 0
                    for gw in _call_sizes(w):
                        slot0_p, ns_p, q_p, _sg_p, off_p = next(call_iter)
                        assert slot0_p == slot_off + g0 and ns_p == gw
                        nc.gpsimd.dma_gather(
                            out_ap=msgs_t[:, g0:g0 + gw, :],
                            in_ap=t_x[c * CHUNK:(c + 1) * CHUNK, :],
                            idxs_ap=idx_ap(off_p, gw * 8),
                            num_idxs=gw * P,
                            num_idxs_reg=gw * P,
                            elem_size=ELEM,
                            single_packet=SINGLE_PACKET,
                            queue_num=q_p,
                        )
                        g0 += gw
                    # per-bin sel stream from HBM (contiguous per partition)
                    sel_t = spool.tile([P, w, P], mybir.dt.float8e4, name="sel")
                    nc.scalar.dma_start(
                        out=sel_t[:].rearrange("r s c -> r (s c)"),
                        in_=t_selm[:, slot_off * P:(slot_off + w) * P],
                    )
                    sidx0 = slot_off
                    for si in range(w):
                        h = int(labels[si])
                        nc.tensor.matmul(
                            pslot(h),
                            lhsT=sel_t[:, si, :],
                            rhs=msgs_t[:, si, :],
                            start=False,
                            stop=False,
                            skip_group_check=True,
                        )
                        if spill[sidx0 + si]:
                            col = int(spill_cols[sidx0 + si])
                            if (sp_state[0] is None
                                    or col >= sp_state[1] + sp_state[2]):
                                take = min(SEL_PIECE, nsp_pad - col)
                                sp_t = sppool.tile(
                                    [P, SEL_PIECE, P], mybir.dt.float8e4,
                                    name="spsel")
                                nc.scalar.dma_start(
                                    out=sp_t[:, :take, :]
                                    .rearrange("r s c -> r (s c)"),
                                    in_=t_selsp[:, col * P:(col + take) * P],
                                )
                                sp_state[:] = [sp_t, col, take]
                            nc.tensor.matmul(
                                pslot(h + 1),
                                lhsT=sp_state[0][:, col - sp_state[1], :],
                                rhs=msgs_t[:, si, :],
                                start=False,
                                stop=False,
                                skip_group_check=True,
                            )
                    slot_off += w

                stage_t = stpool.tile([P, gsg_max, F], mybir.dt.float32, name="stage")
                for j in range(gs):
                    tmp_t = stpool.tile([P, F], mybir.dt.float32, name="tmp", tag="tmp")
                    nc.vector.tensor_copy(out=tmp_t[:], in_=pslot(j)[:, 0:F])
                    nc.vector.tensor_add(
                        out=stage_t[:, j, :],
                        in0=tmp_t[:],
                        in1=pslot(j)[:, F:2 * F],
                    )
                    nc.vector.tensor_tensor(
                        out=stage_t[:, j, :],
                        in0=stage_t[:, j, :],
                        in1=invc_t[:, g_base + j, None].to_broadcast([P, F]),
                        op=mybir.AluOpType.mult,
                    )
                nc.scalar.dma_start(
                    out=t_out[:, g_base * F:(g_base + gs) * F],
                    in_=stage_t[:].rearrange("r g f -> r (g f)")[:, :gs * F],
                )
                g_base += gs

    nc.compile()
    return nc


def kernel(x, edge_idx):
    from concourse.bass_utils import run_bass_kernel_spmd

    (xp, idx_dev, selm_dev, selsp_dev, invc_dev, sched, tot, spill,
     perm) = _host_prep(x, edge_idx)
    nc = _build_program(sched, tot, spill)
    in_maps = [
        {"xp": xp, "idx": idx_dev[k], "selm": selm_dev[k],
         "selsp": selsp_dev[k], "invc": invc_dev[k]}
        for k in range(NCORES)
    ]
    res = run_bass_kernel_spmd(nc, in_maps, list(range(NCORES)))
    return _gather_out(res)[perm]


def _gather_out(res):
    """[P, GPC*F] per-core device layout -> full [NCORES*TPC, F]."""
    return np.concatenate(
        [
            res.results[k]["out"]
            .reshape(P, GPC, F)
            .transpose(1, 0, 2)
            .reshape(TPC, F)
            for k in range(NCORES)
        ],
        axis=0,
    )



# revision 83
# speedup vs baseline: 1.0036x; 1.0036x over previous
"""GNN mean-aggregation message passing on 8 Trainium2 NeuronCores.

out[t] = mean_{e: tgt[e]==t} x[src[e]]   (0 if no incoming edges)

Strategy (target-sharded, uniform SPMD schedule):
  - Targets are dealt to cores serpentine-by-degree so per-(core,group) edge
    counts are balanced; each core owns 12544 output slots (98 groups of 128).
  - Host packs x as bf16 hi|lo pairs -> [N_pad, 128] bf16 (256B rows). The
    hi+lo split recovers ~fp32 precision after the f32 PSUM accumulation.
  - Edges are routed to the owning core, ordered by (supergroup, src-chunk,
    target) and packed into 128-edge slots. Every slot is bound (at compile
    time, uniformly across cores) to a target group g; its edges may only
    reference groups {g, g+1}. One matmul per slot (plus one per spill slot)
    accumulates into per-group PSUM.
  - The one-hot selection matrices are PRECOMPUTED ON HOST (pure index
    metadata) as fp8 and streamed from HBM on the scalar-engine HWDGE ring:
    no on-device is_equal compares at all (frees the DVE).
  - Sources are gathered straight from HBM with dma_gather (int16 indices,
    4 chunks of 25088 rows to fit the int16 range). SWDGE descriptor
    emission (~8ns/idx per Q7 core pair) is the wall, so gathers round-robin
    all 4 SWDGE queues (queue q runs on Q7 pair {2q,2q+1}) for ~4x parallel
    emission; bins are split into equal <=GCALL-slot calls so the ~4-deep
    Pool dispatch window stays in lockstep.
  - Supergroups are 13 groups (4 PSUM banks): consecutive supergroups
    double-buffer in the 8 banks, so finalize never barriers the pipeline.
  - idx is loaded per supergroup (separate tiles: no false WAR dep, few DMA
    sem lanes); out is [partition, group*F] so the store is 16 big
    descriptors per supergroup instead of 128*gs small ones.
  - Finalize per group: (psum_hi + psum_lo) * inv_count -> DMA out.

Measured on the 8-core axon pod: 1642390 ns (baseline) -> ~480000 ns.
"""
import sys

sys.path.insert(0, "/opt/trn_rl_repo")

import numpy as np
import ml_dtypes

bf16 = ml_dtypes.bfloat16

# ---- problem constants (hardcoded per harness contract) ----
N, F, E = 100000, 64, 1600000
P = 128
NCORES = 8
TPC = 12544                  # target slots per core (= 98 * 128)
GPC = TPC // P               # 98 groups per core
NCHUNKS = 4
CHUNK = 25088                # source rows per chunk (< 32768 for int16 idx)
NPAD = NCHUNKS * CHUNK       # 100352 padded source rows
SG_SIZES = [13, 13, 13, 13, 13, 13, 13, 6, 1]   # 4 PSUM banks per sg ->
# consecutive supergroups double-buffer in the 8 banks (no finalize barrier);
# tiny last one keeps the matmul tail short
ELEM = 2 * F                 # 128 bf16 per packed row = 256B
SENT = 384.0                 # sentinel row id for pad edges (no iota match)
SEL_PIECE = 16               # spill-sel window (columns per spill DMA load)
GCALL = 28                   # slots per dma_gather call: bins split across two
                             # queues so per-bin emission latency halves
NQUEUES = 4                  # SWDGE queues; queue q emits on Q7 core pair {2q,2q+1}
SINGLE_PACKET = False        # True (>64-desc packets) faults the DMA engines
DMA_SCRATCH = 16384          # SWDGE ring carveout bytes/partition


def _chunk_order(sched, s):
    return sorted(range(NCHUNKS), key=lambda c: (-len(sched[s][c]), c))


def _call_sizes(w):
    """Split a bin of w slots into equal-ish gather calls of <=GCALL slots."""
    ncalls = -(-w // GCALL)
    base, rem = divmod(w, ncalls)
    return [base + 1] * rem + [base] * (ncalls - rem)


def _call_plan(sched):
    """Gather-call list in emission order: (slot0, nslots, queue, sg).

    Single source of truth for host idx packing and the device program.
    Queue-packed idx layout: queue q only reads partitions [32q, 32q+32),
    so each call's idx lives only in its own queue's partition group.
    Column offsets: per queue, sg0-1 calls pack from col 0; later calls
    pack from col CA (the max sg0-1 extent) so the two idx tiles split
    cleanly at CA with no call straddling.
    """
    plan = []
    so = 0
    q = 0
    for s in range(len(SG_SIZES)):
        for c in _chunk_order(sched, s):
            for gw in _call_sizes(len(sched[s][c])):
                plan.append([so, gw, q % NQUEUES, s])
                q += 1
                so += gw
    offs = [0] * NQUEUES
    colsA = [0] * NQUEUES
    for p in plan:
        if p[3] <= 1:
            p.append(offs[p[2]])
            offs[p[2]] += p[1] * 8
            colsA[p[2]] = offs[p[2]]
    ca = max(colsA)
    offs = [ca] * NQUEUES
    for p in plan:
        if p[3] > 1:
            p.append(offs[p[2]])
            offs[p[2]] += p[1] * 8
    return plan, ca, max(offs)


def _balance_targets(tgt):
    """Serpentine-by-degree target -> (core, rank) assignment.

    Returns perm_o2n[old_target] = new_target_id (core*TPC + rank)."""
    deg = np.bincount(tgt, minlength=N)
    order = np.argsort(-deg, kind="stable")
    perm = np.empty(N, np.int64)
    pos = np.arange(N)
    rows = pos // NCORES
    cols = pos % NCORES
    cores = np.where(rows % 2 == 0, cols, NCORES - 1 - cols)
    for k in range(NCORES):
        ids = order[cores == k]
        perm[ids] = k * TPC + np.arange(ids.size)
    return perm


def _host_prep(x, edge_idx):
    """Build per-core device arrays and the shared slot schedule."""
    x = np.asarray(x, np.float32)
    src = np.asarray(edge_idx[0], np.int64)
    tgt_orig = np.asarray(edge_idx[1], np.int64)

    perm = _balance_targets(tgt_orig)
    tgt = perm[tgt_orig]

    # packed hi|lo bf16 table
    hi = x.astype(bf16)
    lo = (x - hi.astype(np.float32)).astype(bf16)
    xp = np.zeros((NPAD, ELEM), bf16)
    xp[:N, :F] = hi
    xp[:N, F:] = lo

    inv_cnt = np.bincount(tgt, minlength=NCORES * TPC).astype(np.float32)
    inv_cnt = 1.0 / np.maximum(inv_cnt, 1.0)

    core = tgt // TPC
    chunk = src // CHUNK
    gl = (tgt // P) - core * GPC          # local group 0..97
    sg_bounds = np.cumsum([0] + SG_SIZES)
    sg = np.searchsorted(sg_bounds, gl, side="right") - 1
    order = np.lexsort((tgt, chunk, sg, core))

    nsg = len(SG_SIZES)
    bin_id = (core * nsg + sg) * NCHUNKS + chunk
    bin_sizes = np.bincount(bin_id, minlength=NCORES * nsg * NCHUNKS)
    bin_starts = np.zeros(NCORES * nsg * NCHUNKS + 1, np.int64)
    np.cumsum(bin_sizes, out=bin_starts[1:])

    gl_sorted = gl[order]
    tgt_sorted = tgt[order]
    src_sorted = src[order]
    chunk_sorted = chunk[order]

    # ---- build shared schedule: per (sg, c) the block label list ----
    sched = []          # sched[sg][c] = np.array of block labels h (bin-local)
    for s in range(nsg):
        gs = SG_SIZES[s]
        row = []
        for c in range(NCHUNKS):
            e_kh = np.zeros((NCORES, gs), np.int64)
            for k in range(NCORES):
                b = (k * nsg + s) * NCHUNKS + c
                seg = gl_sorted[bin_starts[b]:bin_starts[b + 1]] - sg_bounds[s]
                if seg.size:
                    e_kh[k] = np.bincount(seg, minlength=gs)
            labels = []
            r = e_kh[:, 0].astype(np.int64)
            for h in range(gs):
                s_h = int(np.ceil(r / P).max())
                labels.extend([h] * s_h)
                cap = s_h * P - r
                if h + 1 < gs:
                    r = np.maximum(0, e_kh[:, h + 1] - cap)
                else:
                    assert (cap >= 0).all()
            row.append(np.asarray(labels, np.int64))
        sched.append(row)

    tot_slots = sum(len(row[c]) for row in sched for c in range(NCHUNKS))
    tot = tot_slots * P

    # ---- per-core edge placement into the uniform slot stream ----
    src_local = np.zeros((NCORES, tot), np.int16)
    trow = np.full((NCORES, tot), SENT, np.float32)
    spill = np.zeros(tot_slots, bool)   # slot has any lane in group h+1
    for k in range(NCORES):
        base = 0
        for s in range(nsg):
            for c in _chunk_order(sched, s):
                labels = sched[s][c]
                b = (k * nsg + s) * NCHUNKS + c
                lo_i, hi_i = bin_starts[b], bin_starts[b + 1]
                garr = gl_sorted[lo_i:hi_i] - sg_bounds[s]
                p = 0
                for bi, h in enumerate(labels):
                    upper = np.searchsorted(garr, h + 1, side="right")
                    take = min(P, upper - p)
                    if take > 0:
                        sl = slice(lo_i + p, lo_i + p + take)
                        pos = base + bi * P
                        src_local[k, pos:pos + take] = (
                            src_sorted[sl] - chunk_sorted[sl] * CHUNK
                        ).astype(np.int16)
                        trow[k, pos:pos + take] = (
                            tgt_sorted[sl] % P + P * (garr[p:p + take] - h)
                        ).astype(np.float32)
                        if (garr[p:p + take] > h).any():
                            spill[base // P + bi] = True
                        p += take
                assert p == hi_i - lo_i, (
                    f"core {k} sg {s} c {c}: placed {p} of {hi_i - lo_i}"
                )
                base += len(labels) * P
        assert base == tot

    # device layouts: idx queue-packed — call idx only in its queue's
    # partition group [32q, 32q+32) (16-row wrap, duplicated for the pair's
    # two cores); other groups hold other calls' data (ignored by the ucode)
    plan, ca, ctot = _call_plan(sched)
    idx_dev = []
    for k in range(NCORES):
        arr = np.zeros((P, ctot), np.int16)
        for slot0, ns, q, _sg, off in plan:
            blk = (src_local[k, slot0 * P:(slot0 + ns) * P]
                   .reshape(ns * 8, 16).T)
            arr[32 * q:32 * q + 16, off:off + ns * 8] = blk
            arr[32 * q + 16:32 * q + 32, off:off + ns * 8] = blk
        idx_dev.append(arr)
    # sel one-hot matrices precomputed on host (pure index metadata) and
    # streamed from HBM as fp8: kills the DVE is_equal compares entirely.
    # main: [lane, slot, col] 1.0 where col == tgt%128 within the slot's own
    # group; spill: compact columns, one per spill slot (shared slot set).
    fp8 = ml_dtypes.float8_e4m3
    spill_slots = np.flatnonzero(spill)               # shared across cores
    n_spill = len(spill_slots)
    nsp_pad = (n_spill + SEL_PIECE - 1) // SEL_PIECE * SEL_PIECE
    nslots = tot // P
    lanes = np.tile(np.arange(P), nslots)             # lane of each stream pos
    slots = np.repeat(np.arange(nslots), P)
    selm_dev = []
    selsp_dev = []
    for k in range(NCORES):
        v = trow[k].astype(np.int64)                  # [tot] 0..255 / SENT
        selm = np.zeros((P, nslots, P), fp8)
        m = v < P
        selm[lanes[m], slots[m], v[m]] = 1.0
        sp = np.zeros((P, nsp_pad, P), fp8)
        vs = v.reshape(nslots, P)[spill_slots]        # [n_spill, lane]
        li, si_ = np.nonzero((vs.T >= P) & (vs.T < 2 * P))
        sp[li, si_, vs.T[li, si_] - P] = 1.0
        selm_dev.append(selm.reshape(P, nslots * P))
        selsp_dev.append(sp.reshape(P, nsp_pad * P))
    invc_dev = [
        inv_cnt[k * TPC:(k + 1) * TPC].reshape(GPC, P).T.copy()
        for k in range(NCORES)
    ]
    return (xp, idx_dev, selm_dev, selsp_dev, invc_dev, sched, tot,
            spill, perm)


def _build_program(sched, tot, spill):
    from concourse import bacc, mybir, tile

    nsg = len(SG_SIZES)
    gsg_max = max(SG_SIZES)
    spill_cols = np.cumsum(spill) - spill      # slot -> its trowsp column
    n_spill = int(spill.sum())
    nsp_pad = (n_spill + SEL_PIECE - 1) // SEL_PIECE * SEL_PIECE

    nc = bacc.Bacc(None, target_bir_lowering=False, debug=False,
                   num_swdge_queues=NQUEUES,
                   dynamic_dma_scratch_size=DMA_SCRATCH)
    plan, ca, ctot = _call_plan(sched)
    t_x = nc.dram_tensor("xp", [NPAD, ELEM], mybir.dt.bfloat16, kind="ExternalInput")
    t_idx = nc.dram_tensor("idx", [P, ctot], mybir.dt.int16, kind="ExternalInput")
    t_selm = nc.dram_tensor("selm", [P, (tot // P) * P], mybir.dt.float8e4, kind="ExternalInput")
    t_selsp = nc.dram_tensor("selsp", [P, nsp_pad * P], mybir.dt.float8e4, kind="ExternalInput")
    t_invc = nc.dram_tensor("invc", [P, GPC], mybir.dt.float32, kind="ExternalInput")
    # out laid out [partition, group*F] so the finalize DMA is one contiguous
    # run per partition (16 big descriptors/call instead of 128*gs small ones)
    t_out = nc.dram_tensor("out", [P, GPC * F], mybir.dt.float32, kind="ExternalOutput")

    with tile.TileContext(nc) as tc:
        with (
            tc.tile_pool(name="const", bufs=1) as cpool,
            tc.tile_pool(name="msgs", bufs=8) as mpool,
            tc.tile_pool(name="sel", bufs=3) as spool,
            tc.tile_pool(name="spsel", bufs=2) as sppool,
            tc.tile_pool(name="stage", bufs=2) as stpool,
            tc.tile_pool(name="psum", bufs=8, space="PSUM") as ppool,
        ):
            # idx in two tiles split at column CA (sg0-1 calls | rest):
            # separate tiles avoid a false WAR dep, two loads keep the DMA
            # sem lanes quiet, and the queue-packed layout (each call's idx
            # only in its queue's 32-partition group) makes both tiles 4x
            # smaller than a fully replicated layout.
            idxa_t = cpool.tile([P, ca], mybir.dt.int16)
            idxb_t = cpool.tile([P, ctot - ca], mybir.dt.int16)
            nc.sync.dma_start(out=idxa_t[:], in_=t_idx[:, :ca])
            nc.sync.dma_start(out=idxb_t[:], in_=t_idx[:, ca:])

            def idx_ap(off, width):
                if off + width <= ca:
                    return idxa_t[:, off:off + width]
                assert off >= ca
                return idxb_t[:, off - ca:off - ca + width]

            invc_t = cpool.tile([P, GPC], mybir.dt.float32)
            nc.sync.dma_start(out=invc_t[:], in_=t_invc[:])
            call_iter = iter(plan)

            slot_off = 0     # global slot offset in the stream
            g_base = 0       # global group offset
            sp_state = [None, 0, 0]   # [tile, base_col, width] rolling spill sel
            for s in range(nsg):
                gs = SG_SIZES[s]
                nslots_psum = gs + 1
                nbanks = (nslots_psum + 3) // 4
                pts = [
                    ppool.tile([P, 4 * P], mybir.dt.float32, name=f"ps{s}_{b}", tag="ps")
                    for b in range(nbanks)
                ]
                for pt in pts:
                    nc.vector.memset(pt[:], 0.0)

                def pslot(j):
                    return pts[j // 4][:, (j % 4) * P:(j % 4 + 1) * P]

                for c in _chunk_order(sched, s):
                    labels = sched[s][c]
                    w = len(labels)
                    if w == 0:
                        continue
                    msgs_t = mpool.tile([P, w, ELEM], mybir.dt.bfloat16, name="msgs")
                    # split bins into EQUAL <=GCALL-slot gather calls (equal
                    # sizes keep the ~4-deep Pool dispatch window in lockstep;
                    # much above ~8K idx/call hits SWDGE ring-wrap stalls).
                    # Round-robin the 4 SWDGE queues so descriptor emission
                    # runs on all 4 Q7 core pairs concurrently.
                    g0 = 0
                    for gw in _call_sizes(w):
                        slot0_p, ns_p, q_p, _sg_p, off_p = next(call_iter)
                        assert slot0_p == slot_off + g0 and ns_p == gw
                        nc.gpsimd.dma_gather(
                            out_ap=msgs_t[:, g0:g0 + gw, :],
                            in_ap=t_x[c * CHUNK:(c + 1) * CHUNK, :],
                            idxs_ap=idx_ap(off_p, gw * 8),
                            num_idxs=gw * P,
                            num_idxs_reg=gw * P,
                            elem_size=ELEM,
                            single_packet=SINGLE_PACKET,
                            queue_num=q_p,
                        )
                        g0 += gw
                    # per-bin sel stream from HBM (contiguous per partition)
                    sel_t = spool.tile([P, w, P], mybir.dt.float8e4, name="sel")
                    nc.scalar.dma_start(
                        out=sel_t[:].rearrange("r s c -> r (s c)"),
                        in_=t_selm[:, slot_off * P:(slot_off + w) * P],
                    )
                    sidx0 = slot_off
                    for si in range(w):
                        h = int(labels[si])
                        nc.tensor.matmul(
                            pslot(h),
                            lhsT=sel_t[:, si, :],
                            rhs=msgs_t[:, si, :],
                            start=False,
                            stop=False,
                            skip_group_check=True,
                        )
                        if spill[sidx0 + si]:
                            col = int(spill_cols[sidx0 + si])
                            if (sp_state[0] is None
                                    or col >= sp_state[1] + sp_state[2]):
                                take = min(SEL_PIECE, nsp_pad - col)
                                sp_t = sppool.tile(
                                    [P, SEL_PIECE, P], mybir.dt.float8e4,
                                    name="spsel")
                                nc.scalar.dma_start(
                                    out=sp_t[:, :take, :]
                                    .rearrange("r s c -> r (s c)"),
                                    in_=t_selsp[:, col * P:(col + take) * P],
                                )
                                sp_state[:] = [sp_t, col, take]
                            nc.tensor.matmul(
                                pslot(h + 1),
                                lhsT=sp_state[0][:, col - sp_state[1], :],
                                rhs=msgs_t[:, si, :],
                                start=False,
                                stop=False,
                                skip_group_check=True,
                            )
                    slot_off += w

                stage_t = stpool.tile([P, gsg_max, F], mybir.dt.float32, name="stage")
                for j in range(gs):
                    tmp_t = stpool.tile([P, F], mybir.dt.float32, name="tmp", tag="tmp")
                    nc.vector.tensor_copy(out=tmp_t[:], in_=pslot(j)[:, 0:F])
                    nc.vector.tensor_add(
                        out=stage_t[:, j, :],
                        in0=tmp_t[:],
                        in1=pslot(j)[:, F:2 * F],
                    )
                    nc.vector.tensor_tensor(
                        out=stage_t[:, j, :],
                        in0=stage_t[:, j, :],
                        in1=invc_t[:, g_base + j, None].to_broadcast([P, F]),
                        op=mybir.AluOpType.mult,
                    )
                nc.scalar.dma_start(
                    out=t_out[:, g_base * F:(g_base + gs) * F],
                    in_=stage_t[:].rearrange("r g f -> r (g f)")[:, :gs * F],
                )
                g_base += gs

    nc.compile()
    return nc


def kernel(x, edge_idx):
    from concourse.bass_utils import run_bass_kernel_spmd

    (xp, idx_dev, selm_dev, selsp_dev, invc_dev, sched, tot, spill,
     perm) = _host_prep(x, edge_idx)
    nc = _build_program(sched, tot, spill)
    in_maps = [
        {"xp": xp, "idx": idx_dev[k], "selm": selm_dev[k],
         "selsp": selsp_dev[k], "invc": invc_dev[k]}
        for k in range(NCORES)
    ]
    res = run_bass_kernel_spmd(nc, in_maps, list(range(NCORES)))
    return _gather_out(res)[perm]


def _gather_out(res):
    """[P, GPC*F] per-core device layout -> full [NCORES*TPC, F]."""
    return np.concatenate(
        [
            res.results[k]["out"]
            .reshape(P, GPC, F)
            .transpose(1, 0, 2)
            .reshape(TPC, F)
            for k in range(NCORES)
        ],
        axis=0,
    )

